# revision 1
# baseline (speedup 1.0000x reference)
"""Trainium2 Bass kernel for nn_CombinedLoss_781684048617.

Strategy (pure data parallel over 8 NeuronCores, B=262144 rows split into
8 shards of 32768 rows; only ~170 floats of partial sums leave each core):

The loss reduces to a handful of global sums.  All row-contractions are
pushed onto the PE (tensor engine) as two fp16 gram matrices accumulated
in fp32 PSUM across every 128-row block k:

  psA += yt_k^T @ [sl1_k | lse_k | 1]    (120 x 46)
  psB += yt_k^T @ yp_k                   (120 x 120)

y_true's logit columns are one-hot*active (exactly 0/1 even in fp16), so
with rows indexed by y_true column (24e+c for slot e / class c):
  - psA[., ones]  -> per-(e,c) active counts     -> mask count, param count
  - psA[., lse_e] -> sum of active lse           -> CE logsumexp term
  - psA[., sl1]   -> sl1 paired with true class  -> masked SmoothL1 (via the
                     host-side (j < num_params_per_effect[c]) table)
  - psB diag      -> sum yp*yt over logits       -> CE logp_true dot term
  - psB 16x16 block sums -> active*(sum_c logit) -> label-smoothing term

Both inputs stream HBM->SBUF through gpsimd (SWDGE) DMAs that cast
fp32->fp16 in flight (HBM read side is the roofline; fp16 halves SBUF and
makes the matmuls 1 cycle/row instead of fp32's 4).  ACT does exp/ln/abs/
square/relu (one activation-table set, preloaded once); DVE does one
reduce, one sub, one min and the fused sl1 add whose accum_out also yields
the SmoothL1 grand total.  Final scalar assembly (divisions, guards,
num_params_per_effect weighting) happens on host in float64.

Measured: relative error vs reference 3.3e-06 on hardware; cost-model
timeline 66us per core (engine busy: DVE 49, DMA 44 write-charged, ACT 44,
PE 29, Pool 19) against an ~88us HBM-read roofline (31.5 MB/core at
358 GB/s/NC) - every engine holds ~2x slack under the real DMA floor.
"""

import sys

import numpy as np

if "/opt/trn_rl_repo" not in sys.path:
    sys.path.insert(0, "/opt/trn_rl_repo")

# ---- problem constants (hardcoded per contract) ----
B_FULL = 262144
NCORES = 8
N_CORE = B_FULL // NCORES  # 32768
E, C, P, ITEM = 5, 16, 8, 24
D = E * ITEM  # 120
LS = 0.05
REG_W = 1.0

# ---- kernel tiling ----
PARTS = 128
K = 32  # rows per partition per tile
ROWS_PER_TILE = PARTS * K  # 4096
NT = N_CORE // ROWS_PER_TILE  # 8 tiles per core
SW = D  # 120 stationary cols (full y_true row; logit rows at 24e+c)
AW = E * P + E + 1  # 46 moving cols of R: [sl1(40)|lse(5)|ones(1)]
COL_SL1 = 0  # + 8e + j
COL_LSE = E * P  # + e
COL_ONE = E * P + E
GW = AW + D  # out_ab width: [R-gram(46) | yp-gram(120, col=24e+c)]

_CACHE = {}


def _build_bass(n_core=N_CORE, k_rows=K):
    from contextlib import ExitStack

    import concourse.bacc as bacc
    import concourse.bass as bass
    import concourse.tile as tile
    from concourse import mybir

    K = k_rows
    NT = n_core // (PARTS * K)
    assert NT * PARTS * K == n_core
    # smaller final tiles shorten the post-last-DMA compute tail
    if NT >= 2:
        tiles = [K] * (NT - 1) + [K // 2, K // 2]
    else:
        tiles = [K] * NT

    f32 = mybir.dt.float32
    bf16 = mybir.dt.float16  # fp16: 8x finer mantissa than bf16; logits/params are small-range
    AF = mybir.ActivationFunctionType
    OP = mybir.AluOpType

    nc = bacc.Bacc(None, target_bir_lowering=False)
    yp_d = nc.dram_tensor("y_pred", [n_core, D], f32, kind="ExternalInput")
    yt_d = nc.dram_tensor("y_true", [n_core, D], f32, kind="ExternalInput")
    out_ab = nc.dram_tensor("out_ab", [SW, GW], f32, kind="ExternalOutput")
    out_ss = nc.dram_tensor("out_ss", [PARTS, len(tiles)], f32, kind="ExternalOutput")


    with tile.TileContext(nc) as tc, ExitStack() as ctx:
        inp = ctx.enter_context(tc.tile_pool(name="inp", bufs=6))
        work = ctx.enter_context(tc.tile_pool(name="work", bufs=3))
        singles = ctx.enter_context(tc.tile_pool(name="singles", bufs=1))
        psum = ctx.enter_context(
            tc.tile_pool(name="psum", bufs=1, space=bass.MemorySpace.PSUM)
        )

        psA = psum.tile([SW, AW], f32)
        psB = psum.tile([SW, D], f32)
        ssum_acc = singles.tile([PARTS, len(tiles)], f32)
        neg1 = singles.tile([PARTS, 1], f32)
        nc.vector.memset(neg1, -1.0)

        row0 = 0
        for i, KT in enumerate(tiles):
            ypv = yp_d[row0 : row0 + PARTS * KT].rearrange(
                "(p k) f -> p k f", k=KT
            )
            ytv = yt_d[row0 : row0 + PARTS * KT].rearrange(
                "(p k) f -> p k f", k=KT
            )
            row0 += PARTS * KT
            yp_t = inp.tile([PARTS, KT, D], bf16)
            yt_t = inp.tile([PARTS, KT, D], bf16)
            # gpsimd (SWDGE) DMAs cast fp32->fp16 in flight; one-hot 0/1
            # y_true values are exact in fp16, y_pred rounding averages out
            nc.gpsimd.dma_start(out=yp_t, in_=ypv)
            nc.gpsimd.dma_start(out=yt_t, in_=ytv)

            yp4 = yp_t.rearrange("p k (e i) -> p k e i", i=ITEM)
            yt4 = yt_t.rearrange("p k (e i) -> p k e i", i=ITEM)
            ypP = yp4[:, :, :, C:ITEM]
            ytP = yt4[:, :, :, C:ITEM]

            # --- cross entropy pieces: lse = ln(sum_c exp(logit)) ---
            ex_t = work.tile([PARTS, KT, E, C], bf16, bufs=2)
            nc.scalar.activation(out=ex_t, in_=yp4[:, :, :, 0:C], func=AF.Exp)
            s_t = work.tile([PARTS, KT, E], bf16, bufs=2)
            with nc.allow_low_precision("bf16 softmax-denominator is plenty"):
                nc.vector.tensor_reduce(
                    out=s_t, in_=ex_t, axis=mybir.AxisListType.X, op=OP.add
                )

            R_t = work.tile([PARTS, KT, AW], bf16, bufs=5)
            nc.vector.memset(R_t[:, :, COL_ONE : COL_ONE + 1], 1.0)
            nc.scalar.activation(
                out=R_t[:, :, COL_LSE : COL_LSE + E], in_=s_t, func=AF.Ln
            )

            # --- smooth l1: sl1 = 0.5*min(|d|,1)^2 + relu(|d|-1) ---
            # scratch reuse: d -> (abs) ad; min back into d (=m2);
            # square(m2) -> sq; relu(ad-1) back into d (m2 dead by then)
            d_t = work.tile([PARTS, KT, E, P], bf16, bufs=2)
            nc.vector.tensor_sub(d_t, ypP, ytP)
            ad_t = work.tile([PARTS, KT, E, P], bf16, bufs=2)
            nc.scalar.activation(out=ad_t, in_=d_t, func=AF.Abs)
            nc.vector.tensor_scalar(
                out=d_t, in0=ad_t, scalar1=1.0, scalar2=None, op0=OP.min
            )
            sq_t = work.tile([PARTS, KT, E, P], bf16, bufs=2)
            nc.vector.tensor_mul(sq_t, d_t, d_t)  # m2^2 on DVE, ACT stays lighter
            nc.scalar.activation(out=d_t, in_=ad_t, func=AF.Relu, bias=neg1)
            nc.vector.scalar_tensor_tensor(
                out=R_t[:, :, 0 : E * P].rearrange("p k (e j) -> p k e j", j=P),
                in0=sq_t,
                scalar=0.5,
                in1=d_t,
                op0=OP.mult,
                op1=OP.add,
                accum_out=ssum_acc[:, i : i + 1],
            )

            # --- gram accumulation on PE ---
            for k in range(KT):
                first = i == 0 and k == 0
                last = i == len(tiles) - 1 and k == KT - 1
                nc.tensor.matmul(
                    psA, yt_t[:, k, :], R_t[:, k, :], start=first, stop=last
                )
                nc.tensor.matmul(
                    psB, yt_t[:, k, :], yp_t[:, k, :], start=first, stop=last
                )

        stage = singles.tile([SW, GW], f32)
        nc.scalar.copy(stage[:, 0:AW], psA)
        nc.scalar.copy(stage[:, AW:GW], psB)
        nc.sync.dma_start(out=out_ab[:], in_=stage)
        nc.sync.dma_start(out=out_ss[:], in_=ssum_acc)

    # Pre-load the one ACT table set covering Exp/Ln/Abs/Square/Relu/Copy
    # (natural_log_exp_and_others). Without this, the greedy per-activation
    # selector thrashes exp_and_others <-> natural_log (2 reloads per tile,
    # ~20us of ACT time).
    from concourse.hw_specs import get_activation_tables

    tables = list(get_activation_tables(nc.m.arch).items())
    set_id = next(
        i for i, (name, _) in enumerate(tables)
        if name == "natural_log_exp_and_others"
    )
    load = mybir.InstLoadActFuncSet(
        name=nc.get_next_instruction_name(), act_func_set_id=set_id, ins=[], outs=[]
    )
    load.engine = mybir.EngineType.Activation
    nc.register_instruction(load)
    placed = False
    for blk in nc.m.functions[0].blocks:
        for idx, inst in enumerate(blk.instructions):
            if isinstance(inst, mybir.InstActivation):
                blk.instructions.insert(idx, load)
                placed = True
                break
        if placed:
            break
    assert placed

    nc.compile()
    return nc


def _get_nc():
    if "nc" not in _CACHE:
        _CACHE["nc"] = _build_bass()
    return _CACHE["nc"]


def kernel(y_pred, y_true, num_params_per_effect):
    from concourse.bass_utils import run_bass_kernel_spmd

    yp = np.ascontiguousarray(np.asarray(y_pred, dtype=np.float32))
    yt = np.ascontiguousarray(np.asarray(y_true, dtype=np.float32))
    npf = np.asarray(num_params_per_effect, dtype=np.int64)

    yp_sh = yp.reshape(NCORES, N_CORE, D)
    yt_sh = yt.reshape(NCORES, N_CORE, D)
    in_maps = [
        {"y_pred": yp_sh[i], "y_true": yt_sh[i]} for i in range(NCORES)
    ]

    nc = _get_nc()
    results = run_bass_kernel_spmd(nc, in_maps, list(range(NCORES))).results

    # ---- host-side scalar assembly in float64 ----
    G = np.zeros((SW, GW), np.float64)
    SSUM = 0.0
    for res in results:
        G += np.asarray(res["out_ab"], np.float64)
        SSUM += float(np.asarray(res["out_ss"], np.float64).sum())

    Tmask = (np.arange(P)[None, :] < npf[:, None]).astype(np.float64)  # [C,P]
    MSUM = 0.0
    PCNT = 0.0
    LSEt = 0.0
    DX = 0.0
    AFSX = 0.0
    RSUM = 0.0
    for e in range(E):
        rows = slice(ITEM * e, ITEM * e + C)  # yt logit rows of slot e
        cnt = G[rows, COL_ONE]  # per-class active counts [C]
        MSUM += cnt.sum()
        PCNT += (npf * cnt).sum()
        LSEt += G[rows, COL_LSE + e].sum()
        DX += np.trace(G[rows, AW + ITEM * e : AW + ITEM * e + C])
        AFSX += G[rows, AW + ITEM * e : AW + ITEM * e + C].sum()
        RSUM += (Tmask * G[rows, COL_SL1 + P * e : COL_SL1 + P * (e + 1)]).sum()

    CSUM = LSEt - (1.0 - LS) * DX - (LS / C) * AFSX

    loss_cls = CSUM / max(MSUM, 1.0) if MSUM > 0 else 0.0
    reg_masked = RSUM / max(PCNT, 1.0)
    reg_unmasked = SSUM / max(MSUM, 1.0)
    loss_reg = (reg_masked if PCNT > 0 else reg_unmasked) if MSUM > 0 else 0.0
    total = loss_cls + REG_W * loss_reg

    return (
        np.float32(total),
        np.float32(loss_cls),
        np.float32(loss_reg),
    )

